# revision 1
# baseline (speedup 1.0000x reference)
"""T5-style 6-layer encoder (B=8, S=512, D=768, H=12, DFF=3072) on 8 NeuronCores.

Strategy: data-parallel over batch (1 sequence per core, no collectives).
Host: embedding gather, relative-position bias precompute, weight transposes
(LN weights folded into the following projection weights), bf16 casts, packing
into contiguous [128, X] tiles. Device: whole transformer in feature-major
(d-major) layout; bf16 matmuls with fp32 PSUM accumulation.
"""

import os
import sys

os.environ.setdefault("MYCRO_LOCAL_CACHE", "1")
if "/opt/trn_rl_repo" not in sys.path:
    sys.path.insert(0, "/opt/trn_rl_repo")

import numpy as np
import ml_dtypes

from concourse import bacc, bass, tile, mybir, masks
from concourse.bass_utils import run_bass_kernel_spmd

BF16 = mybir.dt.bfloat16
F32 = mybir.dt.float32
AF = mybir.ActivationFunctionType

L, D, H, DK, DFF = 6, 768, 12, 64, 3072
B, S = 8, 512
N_CORES = 8
DC = D // 128    # 6 d-chunks
FC = DFF // 128  # 24 f-chunks
SC = S // 128    # 4 s-chunks
EPS = 1e-6

_CACHE = {}
LAST_RESULT = None


def _build_nc():
    nc = bacc.Bacc("TRN2", target_bir_lowering=False, debug=False)

    h0t_d = nc.dram_tensor("h0t", (DC, 128, S), F32, kind="ExternalInput")
    bias_d = nc.dram_tensor("bias", (H, SC, 128, S), BF16, kind="ExternalInput")
    qkvw_d = nc.dram_tensor("qkvw", (L, DC, 128, 3 * D), BF16, kind="ExternalInput")
    owt_d = nc.dram_tensor("owt", (L, DC, 128, D), BF16, kind="ExternalInput")
    wiw_d = nc.dram_tensor("wiw", (L, FC, 128, D), BF16, kind="ExternalInput")
    wow_d = nc.dram_tensor("wow", (L, FC, 128, D), BF16, kind="ExternalInput")
    flnw_d = nc.dram_tensor("flnw", (128, DC), F32, kind="ExternalInput")
    out_d = nc.dram_tensor("out", (DC, 128, S), F32, kind="ExternalOutput")

    with tile.TileContext(nc) as tc:
        with tc.tile_pool(name="const", bufs=1) as cpool, \
             tc.tile_pool(name="w", bufs=2) as wpool, \
             tc.tile_pool(name="a", bufs=2) as apool, \
             tc.tile_pool(name="ps", bufs=2, space="PSUM") as pspool:

            ident = cpool.tile([128, 128], BF16, name="ident")
            masks.make_identity(nc, ident[:])
            ones_col = cpool.tile([128, 1], BF16, name="ones_col")
            nc.gpsimd.memset(ones_col[:], 1.0)
            ones_row = cpool.tile([1, 128], BF16, name="ones_row")
            nc.gpsimd.memset(ones_row[:], 1.0)
            ones_row_f = cpool.tile([1, 128], F32, name="ones_row_f")
            nc.gpsimd.memset(ones_row_f[:], 1.0)
            zero_col = cpool.tile([128, 1], F32, name="zero_col")
            nc.gpsimd.memset(zero_col[:], 0.0)
            eps_row = cpool.tile([1, 1], F32, name="eps_row")
            nc.gpsimd.memset(eps_row[:], EPS)
            flnw = cpool.tile([128, DC], F32, name="flnw")
            nc.sync.dma_start(out=flnw[:], in_=flnw_d[:])

            # residual stream, fp32, feature-major: hT[c] = h[c*128:(c+1)*128, :]
            hT = []
            for c in range(DC):
                t = apool.tile([128, S], F32, tag=f"hT{c}", bufs=1, name=f"hT{c}")
                nc.sync.dma_start(out=t[:], in_=h0t_d[c])
                hT.append(t)

            def layer_norm(tag, bufs, bf16_out=True):
                # x[c] = hT[c] * rsqrt(mean_d(hT^2) + eps); LN weight folded into
                # the next projection's weights on host (except final LN).
                ssq = pspool.tile([1, S], F32, tag="acc", bufs=3, name="ssq")
                for c in range(DC):
                    sq = apool.tile([128, S], BF16, tag="sq", bufs=2, name="sq")
                    nc.scalar.activation(sq[:], hT[c][:], AF.Square,
                                         bias=zero_col[:])
                    nc.tensor.matmul(ssq[:], ones_col[:], sq[:],
                                     start=(c == 0), stop=(c == DC - 1))
                std = apool.tile([1, S], F32, tag="std", bufs=1, name="std")
                nc.scalar.activation(std[:], ssq[:], AF.Sqrt,
                                     bias=eps_row[:], scale=1.0 / D)
                inv = apool.tile([1, S], F32, tag="inv", bufs=1, name="inv")
                nc.vector.reciprocal(inv[:], std[:])
                bc = pspool.tile([128, S], F32, tag="acc", bufs=3, name="bc")
                if bf16_out:
                    invb = apool.tile([1, S], BF16, tag="invb", bufs=1, name="invb")
                    nc.vector.tensor_copy(invb[:], inv[:])
                    nc.tensor.matmul(bc[:], ones_row[:], invb[:])
                else:
                    nc.tensor.matmul(bc[:], ones_row_f[:], inv[:])
                dt_out = BF16 if bf16_out else F32
                xs = []
                for c in range(DC):
                    x = apool.tile([128, S], dt_out, tag=tag, bufs=bufs, name=tag)
                    nc.vector.tensor_mul(x[:], hT[c][:], bc[:])
                    xs.append(x)
                return xs

            for l in range(L):
                qkvw = []
                for c in range(DC):
                    t = wpool.tile([128, 3 * D], BF16, tag="qkvw", bufs=7, name="qkvw")
                    nc.sync.dma_start(out=t[:], in_=qkvw_d[l, c])
                    qkvw.append(t)
                owt = []
                for c in range(DC):
                    t = wpool.tile([128, D], BF16, tag="owt", bufs=8, name="owt")
                    nc.sync.dma_start(out=t[:], in_=owt_d[l, c])
                    owt.append(t)

                xT = layer_norm("xT", bufs=7)

                # qT/kT[c] = [128(i), S]; i-chunk c, accumulate over d-chunks
                qT, kT = [], []
                for which, off, outl in ((0, 0, qT), (1, D, kT)):
                    for c in range(DC):
                        ps = pspool.tile([128, S], F32, tag="acc", bufs=3, name="qkps")
                        for d2 in range(DC):
                            nc.tensor.matmul(
                                ps[:],
                                qkvw[d2][:, off + c * 128: off + (c + 1) * 128],
                                xT[d2][:],
                                start=(d2 == 0), stop=(d2 == DC - 1))
                        t = apool.tile([128, S], BF16,
                                       tag=("qT" if which == 0 else "kT"), bufs=7,
                                       name="qkt")
                        nc.vector.tensor_copy(t[:], ps[:])
                        outl.append(t)

                # v[sc] = [128(s), D(i)] natural layout (lhsT for ctx matmul)
                v = []
                for s_ in range(SC):
                    t = apool.tile([128, D], BF16, tag="v", bufs=5, name="v")
                    for half in range(2):
                        ps = pspool.tile([128, 384], F32, tag="acc", bufs=3, name="vps")
                        for d2 in range(DC):
                            nc.tensor.matmul(
                                ps[:],
                                xT[d2][:, s_ * 128:(s_ + 1) * 128],
                                qkvw[d2][:, 2 * D + half * 384: 2 * D + (half + 1) * 384],
                                start=(d2 == 0), stop=(d2 == DC - 1))
                        nc.vector.tensor_copy(t[:, half * 384:(half + 1) * 384], ps[:])
                    v.append(t)

                ctxT = []
                for c in range(DC):
                    t = apool.tile([128, S], BF16, tag="ctxT", bufs=7, name="ctxT")
                    ctxT.append(t)

                for h in range(H):
                    qt, kt = qT[h // 2], kT[h // 2]
                    r0 = (h % 2) * 64
                    attn_tiles = []
                    for c in range(SC):
                        bias_t = apool.tile([128, S], BF16, tag="bias", bufs=5, name="bias_t")
                        nc.sync.dma_start(out=bias_t[:], in_=bias_d[h, c])
                        sc_ps = pspool.tile([128, S], F32, tag="sc", bufs=2, name="sc_ps")
                        nc.tensor.matmul(sc_ps[:],
                                         qt[r0:r0 + 64, c * 128:(c + 1) * 128],
                                         kt[r0:r0 + 64, :])
                        ssb = apool.tile([128, S], F32, tag="ssb", bufs=2, name="ssb")
                        nc.vector.tensor_add(ssb[:], sc_ps[:], bias_t[:])
                        # logits are O(±30) with these weight scales: exp in fp32
                        # is safe without max-subtraction.
                        attn_c = apool.tile([128, S], BF16, tag="attn", bufs=6, name="attn_c")
                        sum_c = apool.tile([128, 1], F32, tag="sum", bufs=6, name="sum_c")
                        nc.scalar.activation(attn_c[:], ssb[:], AF.Exp,
                                             bias=zero_col[:], accum_out=sum_c[:])
                        rec_c = apool.tile([128, 1], F32, tag="rec", bufs=6, name="rec_c")
                        nc.vector.reciprocal(rec_c[:], sum_c[:])
                        nc.vector.tensor_scalar_mul(attn_c[:], attn_c[:], rec_c[:])
                        attn_tiles.append(attn_c)
                    ctx_ps = pspool.tile([64, S], F32, tag="acc", bufs=3, name="ctx_ps")
                    for j in range(SC):
                        tp_ps = pspool.tile([128, S], BF16, tag="tp", bufs=2, name="tp_ps")
                        for c in range(SC):
                            nc.tensor.transpose(tp_ps[:, c * 128:(c + 1) * 128],
                                                attn_tiles[c][:, j * 128:(j + 1) * 128],
                                                ident[:])
                        attnT = apool.tile([128, S], BF16, tag="attnT", bufs=3, name="attnT")
                        nc.vector.tensor_copy(attnT[:], tp_ps[:])
                        nc.tensor.matmul(ctx_ps[:], v[j][:, h * 64:(h + 1) * 64], attnT[:],
                                         start=(j == 0), stop=(j == SC - 1))
                    nc.vector.tensor_copy(ctxT[h // 2][r0:r0 + 64, :], ctx_ps[:])

                # O projection + residual
                for d2 in range(DC):
                    dps = pspool.tile([128, S], F32, tag="acc", bufs=3, name="dps")
                    for ic in range(DC):
                        nc.tensor.matmul(dps[:],
                                         owt[ic][:, d2 * 128:(d2 + 1) * 128],
                                         ctxT[ic][:],
                                         start=(ic == 0), stop=(ic == DC - 1))
                    nc.vector.tensor_add(hT[d2][:], hT[d2][:], dps[:])

                # FFN
                x2T = layer_norm("xT", bufs=7)
                ffs = []
                for f_ in range(FC):
                    wiw = wpool.tile([128, D], BF16, tag="wiw", bufs=3, name="wiw")
                    nc.sync.dma_start(out=wiw[:], in_=wiw_d[l, f_])
                    wow = wpool.tile([128, D], BF16, tag="wow", bufs=25, name="wow")
                    nc.sync.dma_start(out=wow[:], in_=wow_d[l, f_])
                    fps = pspool.tile([128, S], F32, tag="acc", bufs=3, name="fps")
                    for d2 in range(DC):
                        nc.tensor.matmul(fps[:],
                                         wiw[:, d2 * 128:(d2 + 1) * 128],
                                         x2T[d2][:],
                                         start=(d2 == 0), stop=(d2 == DC - 1))
                    ff = apool.tile([128, S], BF16, tag="ff", bufs=25, name="ff")
                    nc.scalar.activation(ff[:], fps[:], AF.Relu,
                                         bias=zero_col[:])
                    ffs.append((ff, wow))
                for d2 in range(DC):
                    d2ps = pspool.tile([128, S], F32, tag="acc", bufs=3, name="d2ps")
                    for f_ in range(FC):
                        ff, wow = ffs[f_]
                        nc.tensor.matmul(d2ps[:],
                                         wow[:, d2 * 128:(d2 + 1) * 128],
                                         ff[:],
                                         start=(f_ == 0), stop=(f_ == FC - 1))
                    nc.vector.tensor_add(hT[d2][:], hT[d2][:], d2ps[:])

            # final layer norm (weight applied explicitly)
            xf = layer_norm("xf", bufs=2, bf16_out=False)
            for c in range(DC):
                o = apool.tile([128, S], F32, tag="outx", bufs=2, name="outx")
                nc.vector.tensor_scalar_mul(o[:], xf[c][:], flnw[:, c:c + 1])
                nc.sync.dma_start(out=out_d[c], in_=o[:])

    nc.compile()
    return nc


def _rel_bucket_np(rel):
    # numpy port of reference.relative_position_bucket (bidirectional, 32/128)
    nb = 16
    ret = (rel > 0).astype(np.int32) * nb
    n = np.abs(rel)
    max_exact = nb // 2
    is_small = n < max_exact
    logf = np.log(np.maximum(n, 1).astype(np.float32) / max_exact)
    val_large = max_exact + (
        logf / np.float32(np.log(128.0 / max_exact)) * (nb - max_exact)
    ).astype(np.int32)
    val_large = np.minimum(val_large, nb - 1)
    return ret + np.where(is_small, n.astype(np.int32), val_large)


def _prep(inputs):
    bf = ml_dtypes.bfloat16
    ids = np.asarray(inputs["input_ids"])
    mask = np.asarray(inputs["attention_mask"], np.float32)
    emb = np.asarray(inputs["embed_w"], np.float32)
    ln1 = np.asarray(inputs["ln1_w"], np.float32)
    qw = np.asarray(inputs["q_w"], np.float32)
    kw = np.asarray(inputs["k_w"], np.float32)
    vw = np.asarray(inputs["v_w"], np.float32)
    relb = np.asarray(inputs["rel_bias_w"], np.float32)
    ow = np.asarray(inputs["o_w"], np.float32)
    ln2 = np.asarray(inputs["ln2_w"], np.float32)
    wiw = np.asarray(inputs["wi_w"], np.float32)
    wow = np.asarray(inputs["wo_w"], np.float32)
    fln = np.asarray(inputs["final_ln_w"], np.float32)

    # shared weights, packed for contiguous [128, X] tile DMAs
    qkvw_p = np.empty((L, DC, 128, 3 * D), dtype=bf)
    owt_p = np.empty((L, DC, 128, D), dtype=bf)
    wiw_p = np.empty((L, FC, 128, D), dtype=bf)
    wow_p = np.empty((L, FC, 128, D), dtype=bf)
    for l in range(L):
        qkv = np.stack([qw[l], kw[l], vw[l]])          # [3, i, d]
        qkv = qkv * ln1[l][None, None, :]              # fold ln1 into q/k/v
        arr = qkv.transpose(2, 0, 1).reshape(D, 3 * D)  # [d, (t,i)]
        qkvw_p[l] = arr.reshape(DC, 128, 3 * D).astype(bf)
        owt_p[l] = ow[l].T.reshape(DC, 128, D).astype(bf)        # [i, d]
        a = (wiw[l] * ln2[l][None, :]).reshape(FC, 128, DC, 128)  # fold ln2
        wiw_p[l] = a.transpose(0, 3, 2, 1).reshape(FC, 128, D).astype(bf)
        wow_p[l] = wow[l].T.reshape(FC, 128, D).astype(bf)       # [f, d]
    flnw_p = np.ascontiguousarray(fln.reshape(DC, 128).T)        # [128, DC]

    # relative position bias, shared table + per-batch mask
    pos = np.arange(S)
    rel = pos[None, :] - pos[:, None]
    bucket = _rel_bucket_np(rel)                       # [S, S]
    pos_bias = relb[bucket]                            # [S, S, H]
    pos_bias = pos_bias.transpose(2, 0, 1)             # [H, Sq, Sk]
    ext = (1.0 - mask) * -1e9                          # [B, S]

    in_maps = []
    for b in range(B):
        bias_b = pos_bias + ext[b][None, None, :]
        bias_p = np.ascontiguousarray(
            bias_b.reshape(H, SC, 128, S)).astype(bf)
        h0 = emb[ids[b]]                               # [S, D]
        h0t = np.ascontiguousarray(h0.T).reshape(DC, 128, S).astype(np.float32)
        in_maps.append({
            "h0t": h0t,
            "bias": bias_p,
            "qkvw": qkvw_p,
            "owt": owt_p,
            "wiw": wiw_p,
            "wow": wow_p,
            "flnw": flnw_p,
        })
    return in_maps


def kernel(**inputs):
    global LAST_RESULT
    if "nc" not in _CACHE:
        _CACHE["nc"] = _build_nc()
    nc = _CACHE["nc"]
    in_maps = _prep(inputs)
    res = run_bass_kernel_spmd(nc, in_maps, core_ids=list(range(N_CORES)))
    LAST_RESULT = res
    outs = [np.asarray(r["out"], np.float32) for r in res.results]
    full = np.stack([o.reshape(D, S).T for o in outs])  # [B, S, D]
    return np.ascontiguousarray(full)

